# revision 39
# baseline (speedup 1.0000x reference)
"""Trainium2 Bass kernel for a pre-LN transformer block (B=4, T=2048, E=1024, H=16).

Sharding: 8 cores = 4 batches x 2 token-halves. Each core receives its batch's
full 2048 tokens (rolled so its own 1024 query tokens come first), computes
K/V for all 2048 tokens (redundantly with its pair core) and everything else
(Q, attention, proj, FFN) only for its own 1024 tokens. Zero cross-core
communication; host reassembles the output.

LayerNorm gains/biases are folded into the matmul weights host-side:
  q = LN1n(x) @ (diag(g1) Wq) + (b1_ln @ Wq)   with LN1n(x) = (x-mu)*rstd
so the device only computes (x-mu)*rstd. All matmuls run in bf16 (fp32
accumulation in PSUM); residuals/softmax stay fp32.
"""

import numpy as np
import ml_dtypes

BF = ml_dtypes.bfloat16
F8 = ml_dtypes.float8_e4m3

B, T, E, H, HS, FF = 4, 2048, 1024, 16, 64, 4096
TQ = T // 2          # own query tokens per core
NCORES = 8
EPS = 1e-5
NMT = T // 128       # 16 token tiles (full batch)
NMQ = TQ // 128      # 8 token tiles (own half)
NJE = E // 128       # 8 feature tiles of E
NJF = FF // 128      # 32 feature tiles of FF

_CACHE = {}
TRACE = False        # set by test harness to capture an NTFF profile
LAST_RESULTS = None  # BassKernelResults from the most recent run


def _build():
    import concourse.bacc as bacc
    import concourse.tile as tile
    from concourse import mybir
    from contextlib import ExitStack

    f32 = mybir.dt.float32
    bf16 = mybir.dt.bfloat16
    f8 = mybir.dt.float8e4
    DR = mybir.MatmulPerfMode.DoubleRow
    AF = mybir.ActivationFunctionType
    OP = mybir.AluOpType

    nc = bacc.Bacc("TRN2", target_bir_lowering=False, debug=False,
                   num_devices=NCORES)

    # ---- DRAM I/O ----
    x_d = nc.declare_dram_parameter("x", [T, E], bf16, isOutput=False)
    wq_d = nc.declare_dram_parameter("wq", [NJE, 128, E], f8, isOutput=False)
    wk_d = nc.declare_dram_parameter("wk", [NJE, 128, E], f8, isOutput=False)
    wv_d = nc.declare_dram_parameter("wv", [NJE, 128, E], f8, isOutput=False)
    wo_d = nc.declare_dram_parameter("wo", [NJE, 128, E], f8, isOutput=False)
    w1_d = nc.declare_dram_parameter("w1", [NJF, 128, E], bf16, isOutput=False)
    w2_d = nc.declare_dram_parameter("w2", [2, 128, NJF // 4, 2, 512], f8, isOutput=False)
    w2b_d = nc.declare_dram_parameter("w2b", [2, 128, NJF // 2, 512], bf16, isOutput=False)
    cq_d = nc.declare_dram_parameter("cq", [128, NJE], f32, isOutput=False)
    ck_d = nc.declare_dram_parameter("ck", [128, NJE], f32, isOutput=False)
    cvb_d = nc.declare_dram_parameter("cvb", [128, E], f32, isOutput=False)
    xq_d = nc.declare_dram_parameter("xq", [TQ, E], f32, isOutput=False)
    b2b_d = nc.declare_dram_parameter("b2b", [128, E], f32, isOutput=False)
    b1c_d = nc.declare_dram_parameter("b1c", [128, NJF], f32, isOutput=False)
    out_d = nc.declare_dram_parameter("out", [TQ, E], f32, isOutput=True)
    rbounce = nc.dram_tensor("rbounce", [H, TQ], f32)

    NSP = NMT // 2       # 8 key-tile pairs
    SC = float(HS) ** -0.5 / 64.0   # wq,wk each carry an 8x scale

    def layernorm(stats_pool, x_sb, out_bf, eps_sb, apply_on_act=True):
        st = stats_pool.tile([128, 2, 6], f32, name="ln_st")
        nc.vector.bn_stats(out=st[:, 0, :], in_=x_sb[:, 0:512])
        nc.vector.bn_stats(out=st[:, 1, :], in_=x_sb[:, 512:1024])
        mv = stats_pool.tile([128, 2], f32, name="ln_mv")
        nc.vector.bn_aggr(out=mv[:], in_=st[:])
        rstd = stats_pool.tile([128, 1], f32, name="ln_rstd")
        nc.scalar.activation(out=rstd[:], in_=mv[:, 1:2], func=AF.Sqrt,
                             bias=eps_sb[:])
        nc.vector.reciprocal(out=rstd[:], in_=rstd[:])
        nmr = stats_pool.tile([128, 1], f32, name="ln_nmr")
        nc.vector.tensor_tensor(out=nmr[:], in0=mv[:, 0:1], in1=rstd[:],
                                op=OP.mult)
        nc.vector.tensor_scalar_mul(out=nmr[:], in0=nmr[:], scalar1=-1.0)
        if apply_on_act:
            nc.scalar.activation(out=out_bf[:], in_=x_sb[:], func=AF.Identity,
                                 bias=nmr[:], scale=rstd[:])
        else:
            # keep the exp stream alone on ACT during attention passes
            nc.vector.tensor_scalar(out=out_bf[:], in0=x_sb[:],
                                    scalar1=rstd[:], scalar2=nmr[:],
                                    op0=OP.mult, op1=OP.add)

    with tile.TileContext(nc) as tc:
        top = ExitStack()

        # ---- constants + long-lived weights (left stack bottom) ----
        const = top.enter_context(tc.tile_pool(name="const", bufs=1, side="left"))
        xin = top.enter_context(tc.tile_pool(name="ln1x", bufs=6, side="left"))
        xt = {}

        def load_x(mt):
            # issued from inside the LN1 loop at a fixed prefetch distance:
            # issuing all 16 up-front head-blocks the sync queue (and the
            # transposes behind it) on the pool-recycle anti-deps
            x_sb = xin.tile([128, E], bf16)
            nc.sync.dma_start(out=x_sb[:], in_=x_d[mt * 128:(mt + 1) * 128, :])
            xt[mt] = x_sb
        for mt in range(4):
            load_x(mt)
        eps_sb = const.tile([128, 1], f32)
        nc.vector.memset(eps_sb[:], EPS)
        cq_sb = const.tile([128, NJE], f32)
        nc.sync.dma_start(out=cq_sb[:], in_=cq_d[:])
        ck_sb = const.tile([128, NJE], f32)
        nc.sync.dma_start(out=ck_sb[:], in_=ck_d[:])

        b2_sb = const.tile([128, E], f32)
        nc.sync.dma_start(out=b2_sb[:], in_=b2b_d[:])
        b1_sb = const.tile([128, NJF], f32)
        nc.sync.dma_start(out=b1_sb[:], in_=b1c_d[:])
        zero_sb = const.tile([128, 512], bf16)
        nc.vector.memset(zero_sb[:], 0.0)
        from concourse import masks
        ident = const.tile([128, 128], bf16)
        masks.make_identity(nc, ident[:])
        ones64 = const.tile([1, 64], f32)
        nc.vector.memset(ones64[:], 1.0)
        wo_sb = const.tile([128, NJE, E], f8)
        for j in range(NJE):
            nc.gpsimd.dma_start(out=wo_sb[:, j, :], in_=wo_d[j])

        # qkv weights: issued up-front from a pre-allocated pool so the loads
        # carry no SBUF-reuse anti-dependency on the LN1 pipeline
        w_es = ExitStack()
        wqkv = w_es.enter_context(tc.tile_pool(name="w_pool", bufs=1, side="left"))
        wq_sb = wqkv.tile([128, NJE, E], f8, name="wq")
        wk_sb = wqkv.tile([128, NJE, E], f8, name="wk")
        wv_sb = wqkv.tile([128, NJE, E], f8, name="wv")
        cv_sb = wqkv.tile([128, E], f32, name="cv")
        nc.sync.dma_start(out=cv_sb[:], in_=cvb_d[:])
        for j in range(NJE):
            nc.gpsimd.dma_start(out=wk_sb[:, j, :], in_=wk_d[j])
        for j in range(NJE):
            nc.gpsimd.dma_start(out=wq_sb[:, j, :], in_=wq_d[j])
        for j in range(NJE):
            nc.gpsimd.dma_start(out=wv_sb[:, j, :], in_=wv_d[j])

        # ---------- LN1 (transpose on the PE; emitted per-tile, woven into
        # pass A so the exp stream starts as soon as the first LN1 group
        # lands; a DMA-transpose in the per-tile chain serializes the DVE
        # queue behind multi-us DMA latencies) ----------
        hT_es = ExitStack()
        hT_pool = hT_es.enter_context(tc.tile_pool(name="hT", bufs=1, side="left"))
        h8 = [hT_pool.tile([128, NJE, 4, 128], f8, name=f"h8{g}")
              for g in range(4)]
        ln1_es = ExitStack()
        stp = ln1_es.enter_context(tc.tile_pool(name="ln1s", bufs=10, side="left"))
        hbp = ln1_es.enter_context(tc.tile_pool(name="ln1h", bufs=3, side="left"))

        # ---------- persistent QKV activations (right stack) ----------
        oT = top.enter_context(tc.tile_pool(name="oT", bufs=1, side="right")) \
            .tile([128, NJE, TQ], f8)
        qkv_es = ExitStack()
        qkv_pool = qkv_es.enter_context(
            tc.tile_pool(name="qkvact", bufs=1, side="right"))
        # q^T zero-padded per head (see scores matmul): head h in rows
        # (h%2)*64..+64 of [:, h, :], other 64 rows zero.
        # q/k live in f8: the scores matmul runs fp8 at the same 1 cy/col as
        # bf16, and the ~1% logit noise is far inside the error budget, so
        # this is 32KB of SBUF for free.
        qT = qkv_pool.tile([128, H, TQ], f8)
        kT = qkv_pool.tile([128, NJE, T], f8)
        # v (8x via wv scale) + 0.5-ones column: softmax sum = 0.5*sum(att),
        # so normalized attention lands at 16x natural scale (f8-friendly);
        # the proj epilogue folds in 1/128 (16x and the 8x wo scale).
        v_aug = qkv_pool.tile([128, NSP, 2, H, HS + 1], f8)
        for hh in range(H):
            p0 = 64 - (hh % 2) * 64
            nc.gpsimd.memset(qT[p0:p0 + 64, hh, :], 0.0)
        nc.vector.memset(v_aug[:, :, :, :, HS:HS + 1], 0.5)

        work_es = ExitStack()
        wps = work_es.enter_context(
            tc.tile_pool(name="work_ps", bufs=2, space="PSUM"))

        def emit_ln1(mt):
            if mt + 4 < NMT:
                load_x(mt + 4)
            h_bf = hbp.tile([128, E], bf16)
            layernorm(stp, xt[mt], h_bf, eps_sb)
            trp = wps.tile([128, E], bf16, name="ps_w")
            for j in range(NJE):
                nc.tensor.transpose(trp[:, j * 128:(j + 1) * 128],
                                    h_bf[:, j * 128:(j + 1) * 128], ident[:])
            g, sl = mt // 4, mt % 4
            nc.vector.tensor_copy(
                out=h8[g][:, 0:4, sl, :],
                in_=trp[:, 0:512].rearrange("p (j c) -> p j c", j=4))
            nc.scalar.activation(
                out=h8[g][:, 4:8, sl, :],
                in_=trp[:, 512:1024].rearrange("p (j c) -> p j c", j=4),
                func=AF.Copy)

        def emit_q(g, mf, pool):
                pq = pool.tile([128, 512], f32, name="ps_w")
                for j in range(0, NJE, 2):
                    nc.tensor.matmul(pq[:], wq_sb[:, j:j + 2, mf * 128:(mf + 1) * 128],
                                     h8[g][:, j:j + 2, :, :], perf_mode=DR,
                                     start=(j == 0), stop=(j == NJE - 2))
                sl = slice(g * 512, (g + 1) * 512)
                nc.vector.tensor_scalar_add(out=qT[0:64, 2 * mf, sl],
                                            in0=pq[0:64, :],
                                            scalar1=cq_sb[0:64, mf:mf + 1])
                nc.vector.tensor_scalar_add(out=qT[64:128, 2 * mf + 1, sl],
                                            in0=pq[64:128, :],
                                            scalar1=cq_sb[64:128, mf:mf + 1])

        def emit_k(g, mf, pool):
            pk = pool.tile([128, 512], f32, name="ps_w")
            for j in range(0, NJE, 2):
                nc.tensor.matmul(pk[:], wk_sb[:, j:j + 2, mf * 128:(mf + 1) * 128],
                                 h8[g][:, j:j + 2, :, :], perf_mode=DR,
                                 start=(j == 0), stop=(j == NJE - 2))
            nc.vector.tensor_scalar_add(out=kT[:, mf, g * 512:(g + 1) * 512],
                                        in0=pk[:],
                                        scalar1=ck_sb[:, mf:mf + 1])

        def emit_v(st, pool):
            pv0 = pool.tile([128, 512], f32, name="ps_w")
            pv1 = pool.tile([128, 512], f32, name="ps_w")
            for j in range(0, NJE, 2):
                lhsT = h8[st // 4][:, j:j + 2, st % 4, :]
                nc.tensor.matmul(pv0[:], lhsT, wv_sb[:, j:j + 2, 0:512],
                                 perf_mode=DR, start=(j == 0), stop=(j == NJE - 2))
                nc.tensor.matmul(pv1[:], lhsT, wv_sb[:, j:j + 2, 512:1024],
                                 perf_mode=DR, start=(j == 0), stop=(j == NJE - 2))
            nc.vector.tensor_tensor(
                out=v_aug[:, st // 2, st % 2, 0:8, 0:HS],
                in0=pv0.rearrange("p (h d) -> p h d", h=8),
                in1=cv_sb[:, 0:512].rearrange("p (h d) -> p h d", h=8),
                op=OP.add)
            nc.vector.tensor_tensor(
                out=v_aug[:, st // 2, st % 2, 8:16, 0:HS],
                in0=pv1.rearrange("p (h d) -> p h d", h=8),
                in1=cv_sb[:, 512:1024].rearrange("p (h d) -> p h d", h=8),
                op=OP.add)

        # Minimal pre-attention block. Scores consume kT slices in st order,
        # so head 0 only needs K(g0,mf0) before its first scores; later LN1
        # groups and their dependent K/V blocks weave into pass A.
        for mt in range(4):
            emit_ln1(mt)
        emit_k(0, 0, wps)
        emit_q(0, 0, wps)
        emit_v(0, wps)
        emit_v(1, wps)

        # ---------- attention (+ woven filler work) ----------
        att_es = ExitStack()
        aps = att_es.enter_context(tc.tile_pool(name="att_ps", bufs=2, space="PSUM"))
        ops = att_es.enter_context(tc.tile_pool(name="att_po", bufs=2, space="PSUM"))
        atp = att_es.enter_context(tc.tile_pool(name="att_t", bufs=2, side="right"))
        rp = att_es.enter_context(tc.tile_pool(name="att_r", bufs=4, side="right"))
        rbp = att_es.enter_context(tc.tile_pool(name="att_rb", bufs=2, side="right"))

        po_live = {}
        att_live = {}

        def emit_scores(p, h, sp):
            at2 = atp.tile([128, 2, 512], f8, name="att")
            ps = aps.tile([128, 2, 512], f32, name="ps_sc")
            qsl = qT[:, h, p * 512:(p + 1) * 512]
            for k2 in range(2):
                st = 2 * sp + k2
                nc.tensor.matmul(ps[:, k2, :],
                                 kT[:, h // 2, st * 128:(st + 1) * 128],
                                 qsl, start=True, stop=True)
            nc.scalar.activation(out=at2[:], in_=ps[:], func=AF.Exp, scale=SC)
            att_live[(h, sp)] = at2

        def emit_av(p, h, sp):
            if sp == 0:
                po_live[h] = ops.tile([HS + 1, 512], f32, name="ps_o")
            po = po_live[h]
            at2 = att_live.pop((h, sp))
            nc.tensor.matmul(po[:], v_aug[:, sp, :, h, :], at2[:],
                             perf_mode=DR,
                             start=(sp == 0), stop=(sp == NSP - 1))
            if sp == NSP - 1:
                emit_head_finish(p, h, po_live.pop(h))

        def emit_head_finish(p, h, po):
            # Copy the unnormalized head out of PSUM immediately (frees the
            # accumulator for head h+2) and broadcast 1/sum across the 64
            # partitions with a K=1 matmul on the tensor engine — a
            # DRAM-bounce broadcast here held po/att2 ~10us per head and
            # throttled the whole exp pipeline.
            sl = slice(p * 512, (p + 1) * 512)
            s1 = rp.tile([1, 512], f32, name="ssum")
            nc.vector.tensor_copy(out=s1[:], in_=po[HS:HS + 1, :])
            o_un = rbp.tile([64, 512], f32, name="oun")
            nc.vector.tensor_copy(out=o_un[:], in_=po[0:HS, :])
            r1 = rp.tile([1, 512], f32, name="rsum")
            nc.vector.reciprocal_approx_fast(out=r1[:], in_=s1[:])
            rb = wps.tile([64, 512], f32, name="ps_w")
            nc.tensor.matmul(rb[:], ones64[:], r1[:], start=True, stop=True)
            p0 = (h % 2) * 64
            nc.vector.tensor_tensor(out=oT[p0:p0 + 64, h // 2, sl],
                                    in0=o_un[:], in1=rb[:], op=OP.mult)

        def run_pass(p, fillers):
            # fillers: list of (idx, closure) woven in after step idx
            fmap = {}
            for idx, fn in fillers:
                fmap.setdefault(idx, []).append(fn)
            steps = [(h, sp) for h in range(H) for sp in range(NSP)]
            for i, (h, sp) in enumerate(steps):
                emit_scores(p, h, sp)
                if i > 0:
                    emit_av(p, *steps[i - 1])
                for fn in fmap.get(i, []):
                    fn()
            emit_av(p, *steps[-1])
            for fn in fmap.get(len(steps), []):
                fn()

        # pass-A fillers: LN1 tiles, V pairs and K(g*,0) woven just ahead of
        # head 0's consumption; K(mf)+Q(g0,mf) during head 2mf-1; Q(g1,*)
        # (pass-B only) late.
        fillA = [(0, lambda: emit_ln1(4)), (0, lambda: emit_ln1(5)),
                 (0, lambda: emit_v(2, wps)), (0, lambda: emit_v(3, wps)),
                 (1, lambda: emit_ln1(6)), (1, lambda: emit_ln1(7)),
                 (1, lambda: emit_k(1, 0, wps)),
                 (2, lambda: emit_ln1(8)), (2, lambda: emit_ln1(9)),
                 (2, lambda: emit_v(4, wps)), (2, lambda: emit_v(5, wps)),
                 (3, lambda: emit_ln1(10)), (3, lambda: emit_ln1(11)),
                 (3, lambda: emit_v(6, wps)), (3, lambda: emit_v(7, wps)),
                 (3, lambda: emit_k(2, 0, wps)),
                 (4, lambda: emit_ln1(12)), (4, lambda: emit_ln1(13)),
                 (4, lambda: emit_v(8, wps)), (4, lambda: emit_v(9, wps)),
                 (5, lambda: emit_ln1(14)), (5, lambda: emit_ln1(15)),
                 (5, lambda: emit_k(3, 0, wps)),
                 (5, lambda: emit_v(10, wps)), (5, lambda: emit_v(11, wps)),
                 (6, lambda: emit_v(12, wps)), (6, lambda: emit_v(13, wps)),
                 (7, lambda: emit_v(14, wps)), (7, lambda: emit_v(15, wps))]
        for mf in range(1, NJE):
            base = 8 * (2 * mf - 1)
            for g in range(4):
                fillA.append((base + 2 * g,
                              (lambda gg, m: lambda: emit_k(gg, m, wps))(g, mf)))
            fillA.append((base + 7, (lambda m: lambda: emit_q(0, m, wps))(mf)))
        for mf in range(NJE):
            fillA.append((100 + 3 * mf, (lambda m: lambda: emit_q(1, m, wps))(mf)))
        run_pass(0, fillA)
        ln1_es.close()
        hT_es.close()
        w_es.close()

        # ---------- proj + LN2 + FFN emitters ----------
        ffn_es = ExitStack()
        pxp = ffn_es.enter_context(tc.tile_pool(name="proj_x", bufs=1, side="left"))
        xrp = ffn_es.enter_context(tc.tile_pool(name="xr", bufs=8, side="left"))
        h2p = ffn_es.enter_context(tc.tile_pool(name="h2T", bufs=1, side="left"))
        f1p8 = ffn_es.enter_context(tc.tile_pool(name="ffnT8", bufs=1, side="left"))
        f1pb = ffn_es.enter_context(tc.tile_pool(name="ffnTb", bufs=1, side="left"))
        w1p = ffn_es.enter_context(tc.tile_pool(name="f1w", bufs=2, side="left"))
        w2p8 = ffn_es.enter_context(tc.tile_pool(name="f2w8", bufs=1, side="left"))
        w2pb = ffn_es.enter_context(tc.tile_pool(name="f2wb", bufs=1, side="left"))
        stp2 = ffn_es.enter_context(tc.tile_pool(name="ln2s", bufs=6, side="left"))
        hbp2 = ffn_es.enter_context(tc.tile_pool(name="ln2h", bufs=1, side="left"))
        f2op = ffn_es.enter_context(tc.tile_pool(name="f2o", bufs=2, side="left"))
        # per-pass activation tiles cycle through bufs=1 pools: pass B reuses
        # pass A's buffer once the pass-A FFN (woven into attention pass B)
        # has consumed it
        h2Tb, ffnT8, ffnTb = {}, {}, {}
        xr_t = {}

        def emit_proj_ln2(p, mtl):
            if mtl == 0:
                h2Tb[p] = h2p.tile([128, NJE, 4, 128], bf16, name="h2T")
            mt = p * 4 + mtl
            x_sb = pxp.tile([128, E], f32, name="xq")
            nc.sync.dma_start(out=x_sb[:], in_=xq_d[mt * 128:(mt + 1) * 128, :])
            pa = wps.tile([128, 512], f32, name="ps_w")
            pb = wps.tile([128, 512], f32, name="ps_w")
            for j in range(0, NJE, 2):
                lhsT = oT[:, j:j + 2, mt * 128:(mt + 1) * 128]
                nc.tensor.matmul(pa[:], lhsT, wo_sb[:, j:j + 2, 0:512],
                                 perf_mode=DR, start=(j == 0), stop=(j == NJE - 2))
                nc.tensor.matmul(pb[:], lhsT, wo_sb[:, j:j + 2, 512:1024],
                                 perf_mode=DR, start=(j == 0), stop=(j == NJE - 2))
            xr = xrp.tile([128, E], f32, name="xr")
            xr_t[mt] = xr
            nc.vector.scalar_tensor_tensor(
                out=xr[:, 0:512], in0=pa[:], scalar=1.0 / 128.0,
                in1=x_sb[:, 0:512], op0=OP.mult, op1=OP.add)
            nc.vector.scalar_tensor_tensor(
                out=xr[:, 512:1024], in0=pb[:], scalar=1.0 / 128.0,
                in1=x_sb[:, 512:1024], op0=OP.mult, op1=OP.add)
            h_bf = hbp2.tile([128, E], bf16)
            layernorm(stp2, xr[:], h_bf, eps_sb, apply_on_act=False)
            nc.sync.dma_start_transpose(out=h2Tb[p][:, :, mtl, :], in_=h_bf[:])
            # after LN2 consumed xr, fold the output bias in place so the
            # FFN2 epilogue is a single op
            nc.vector.tensor_tensor(out=xr[:], in0=xr[:], in1=b2_sb[:],
                                    op=OP.add)

        def emit_ffn1(p, mf0, nmf):
            if mf0 == 0:
                ffnT8[p] = f1p8.tile([128, NJF // 2, 512], f8, name="fT8")
                ffnTb[p] = f1pb.tile([128, NJF // 2, 512], bf16, name="fTb")
            for mf in range(mf0, mf0 + nmf):
                w1_sb = w1p.tile([128, NJE, 128], bf16, name="w1t")
                nc.gpsimd.dma_start(
                    out=w1_sb[:],
                    in_=w1_d[mf].rearrange("p (j c) -> p j c", j=NJE))
                pf = wps.tile([128, 512], f32, name="ps_w")
                for j in range(NJE):
                    nc.tensor.matmul(pf[:], w1_sb[:, j, :],
                                     h2Tb[p][:, j, :, :],
                                     start=(j == 0), stop=(j == NJE - 1))
                dst = ffnT8[p][:, mf, :] if mf < NJF // 2 else \
                    ffnTb[p][:, mf - NJF // 2, :]
                # W1 carries the 8x scale (exact in bf16), so this single DVE
                # op yields 8x the true hidden without touching ACT
                nc.vector.scalar_tensor_tensor(
                    out=dst, in0=pf[:], scalar=b1_sb[:, mf:mf + 1],
                    in1=zero_sb[:], op0=OP.add, op1=OP.max)

        def emit_ffn2_w(nbh):
            w2_sb = w2p8.tile([128, NJF // 4, 2, 512], f8, name="w2t")
            nc.gpsimd.dma_start(out=w2_sb[:], in_=w2_d[nbh])
            w2b_sb = w2pb.tile([128, NJF // 2, 512], bf16, name="w2bt")
            nc.gpsimd.dma_start(out=w2b_sb[:], in_=w2b_d[nbh])
            return w2_sb, w2b_sb

        def emit_ffn2(p, nbh, w2_sb, w2b_sb, tps, mtls=(0, 1, 2, 3)):
            psums = {}
            for mtl in mtls:
                mt = p * 4 + mtl
                psums[mt] = tps.tile([128, 512], f32, name="ps_w")
            for kp in range(NJF // 4):
                for mtl in mtls:
                    mt = p * 4 + mtl
                    nc.tensor.matmul(psums[mt][:],
                                     ffnT8[p][:, 2 * kp:2 * kp + 2, mtl * 128:(mtl + 1) * 128],
                                     w2_sb[:, kp, :, :], perf_mode=DR,
                                     start=(kp == 0), stop=False)
            for k in range(NJF // 2):
                for mtl in mtls:
                    mt = p * 4 + mtl
                    nc.tensor.matmul(psums[mt][:],
                                     ffnTb[p][:, k, mtl * 128:(mtl + 1) * 128],
                                     w2b_sb[:, k, :],
                                     start=False, stop=(k == NJF // 2 - 1))
            for mt, ps2 in psums.items():
                o_sb = f2op.tile([128, 512], f32, name="osb")
                nc.vector.scalar_tensor_tensor(
                    out=o_sb[:], in0=ps2[:], scalar=1.0 / 64.0,
                    in1=xr_t[mt][:, nbh * 512:(nbh + 1) * 512],
                    op0=OP.mult, op1=OP.add)
                nc.sync.dma_start(
                    out=out_d[mt * 128:(mt + 1) * 128,
                              nbh * 512:(nbh + 1) * 512],
                    in_=o_sb[:])

        # pass B with pass-A proj/LN2/FFN woven in
        fillB = []
        for mtl in range(4):
            fillB.append((2 + 3 * mtl, (lambda m: lambda: emit_proj_ln2(0, m))(mtl)))
        for c in range(16):
            fillB.append((16 + 3 * c,
                          (lambda c0: lambda: emit_ffn1(0, 2 * c0, 2))(c)))
        w2h = {}

        def load_w2h(nbh):
            w2h[nbh] = emit_ffn2_w(nbh)
        fillB.append((88, lambda: load_w2h(0)))
        fillB.append((96, lambda: emit_ffn2(0, 0, *w2h[0], wps, (0, 1))))
        fillB.append((104, lambda: emit_ffn2(0, 0, *w2h[0], wps, (2, 3))))
        fillB.append((108, lambda: load_w2h(1)))
        fillB.append((114, lambda: emit_ffn2(0, 1, *w2h[1], wps, (0, 1))))
        fillB.append((121, lambda: emit_ffn2(0, 1, *w2h[1], wps, (2, 3))))
        run_pass(1, fillB)
        att_es.close()
        qkv_es.close()

        # ---------- tail ----------
        # proj/LN2-B first so their DVE/ACT chains run under FFN2-A's
        # matmuls; FFN1-B then finds h2Tb ready.
        tail_es = ExitStack()
        tps = tail_es.enter_context(tc.tile_pool(name="f2ps", bufs=4, space="PSUM"))
        for mtl in range(4):
            emit_proj_ln2(1, mtl)
        emit_ffn1(1, 0, NJF)
        for nbh in range(2):
            w2_sb, w2b_sb = emit_ffn2_w(nbh)
            emit_ffn2(1, nbh, w2_sb, w2b_sb, tps)

        tail_es.close()
        ffn_es.close()
        work_es.close()
        top.close()

    nc.compile()
    return nc


def _prep_weights(ln1_g, ln1_b, Wq, Wk, Wv, Wo, bo, ln2_g, ln2_b, W1, b1, W2, b2):
    f64 = np.float64
    g1 = np.asarray(ln1_g, f64)
    b1ln = np.asarray(ln1_b, f64)
    g2 = np.asarray(ln2_g, f64)
    b2ln = np.asarray(ln2_b, f64)

    def flat_qkv(W):
        return np.asarray(W, f64).transpose(1, 0, 2).reshape(E, H * HS)

    Wqf, Wkf, Wvf = flat_qkv(Wq), flat_qkv(Wk), flat_qkv(Wv)
    out = {}
    # All f8 weights carry an 8x scale so sigma~0.02 values clear the e4m3
    # subnormal floor; the kernel folds the compensating scales into the exp
    # (1/64), the proj epilogue (1/128, including the 16x from the 0.5 ones
    # column), and the FFN2 epilogue (1/64).
    out["wq"] = np.ascontiguousarray((8 * g1[:, None] * Wqf).reshape(NJE, 128, E).astype(F8))
    out["wk"] = np.ascontiguousarray((8 * g1[:, None] * Wkf).reshape(NJE, 128, E).astype(F8))
    out["wv"] = np.ascontiguousarray((8 * g1[:, None] * Wvf).reshape(NJE, 128, E).astype(F8))
    cq = (8 * b1ln @ Wqf).astype(np.float32)
    ck = (8 * b1ln @ Wkf).astype(np.float32)
    cv = (8 * b1ln @ Wvf).astype(np.float32)
    out["cq"] = np.ascontiguousarray(cq.reshape(NJE, 128).T)
    out["ck"] = np.ascontiguousarray(ck.reshape(NJE, 128).T)
    out["cvb"] = np.ascontiguousarray(np.broadcast_to(cv, (128, E)))
    out["wo"] = np.ascontiguousarray(
        (8 * np.asarray(Wo, f64)).reshape(NJE, 128, E).astype(F8))
    W1p = 8 * g2[:, None] * np.asarray(W1, f64)
    b1p = (8 * (np.asarray(b1, f64) + b2ln @ np.asarray(W1, f64))).astype(np.float32)
    out["w1"] = np.ascontiguousarray(
        W1p.reshape(NJE, 128, NJF, 128).transpose(2, 1, 0, 3).reshape(NJF, 128, E).astype(BF))
    out["b1c"] = np.ascontiguousarray(b1p.reshape(NJF, 128).T)
    w2s = (8 * np.asarray(W2, f64)).reshape(NJF, 128, 2, 512)
    out["w2"] = np.ascontiguousarray(
        w2s[:NJF // 2].reshape(NJF // 4, 2, 128, 2, 512)
        .transpose(3, 2, 0, 1, 4).astype(F8))
    out["w2b"] = np.ascontiguousarray(
        w2s[NJF // 2:].transpose(2, 1, 0, 3).astype(BF))
    out["b2b"] = np.ascontiguousarray(
        np.broadcast_to(np.asarray(b2, np.float32), (128, E)))
    return out


def kernel(x, ln1_g, ln1_b, Wq, Wk, Wv, Wo, bo, ln2_g, ln2_b, W1, b1, W2, b2):
    global LAST_RESULTS
    from concourse.bass_utils import run_bass_kernel_spmd

    if "nc" not in _CACHE:
        _CACHE["nc"] = _build()
    nc = _CACHE["nc"]

    wmap = _prep_weights(ln1_g, ln1_b, Wq, Wk, Wv, Wo, bo,
                         ln2_g, ln2_b, W1, b1, W2, b2)
    x = np.asarray(x, np.float32)

    in_maps = []
    for c in range(NCORES):
        b, half = c // 2, c % 2
        xb = x[b]
        x_roll = np.ascontiguousarray(
            np.concatenate([xb[half * TQ:], xb[:half * TQ]], axis=0))
        m = dict(wmap)
        m["x"] = x_roll.astype(BF)
        m["xq"] = np.ascontiguousarray(
            x_roll[:TQ] + np.asarray(bo, np.float32)[None, :])
        in_maps.append(m)

    res = run_bass_kernel_spmd(nc, in_maps, list(range(NCORES)), trace=TRACE)
    LAST_RESULTS = res

    out = np.empty((B, T, E), np.float32)
    for c in range(NCORES):
        b, half = c // 2, c % 2
        out[b, half * TQ:(half + 1) * TQ] = res.results[c]["out"]
    return out



# revision 40
# speedup vs baseline: 1.0650x; 1.0650x over previous
"""Trainium2 Bass kernel for a pre-LN transformer block (B=4, T=2048, E=1024, H=16).

Sharding: 8 cores = 4 batches x 2 token-halves. Each core receives its batch's
full 2048 tokens (rolled so its own 1024 query tokens come first), computes
K/V for all 2048 tokens (redundantly with its pair core) and everything else
(Q, attention, proj, FFN) only for its own 1024 tokens. Zero cross-core
communication; host reassembles the output.

LayerNorm gains/biases are folded into the matmul weights host-side:
  q = LN1n(x) @ (diag(g1) Wq) + (b1_ln @ Wq)   with LN1n(x) = (x-mu)*rstd
so the device only computes (x-mu)*rstd. All matmuls run in bf16 (fp32
accumulation in PSUM); residuals/softmax stay fp32.
"""

import numpy as np
import ml_dtypes

BF = ml_dtypes.bfloat16
F8 = ml_dtypes.float8_e4m3

B, T, E, H, HS, FF = 4, 2048, 1024, 16, 64, 4096
TQ = T // 2          # own query tokens per core
NCORES = 8
EPS = 1e-5
NMT = T // 128       # 16 token tiles (full batch)
NMQ = TQ // 128      # 8 token tiles (own half)
NJE = E // 128       # 8 feature tiles of E
NJF = FF // 128      # 32 feature tiles of FF

_CACHE = {}
TRACE = False        # set by test harness to capture an NTFF profile
LAST_RESULTS = None  # BassKernelResults from the most recent run


def _build():
    import concourse.bacc as bacc
    import concourse.tile as tile
    from concourse import mybir
    from contextlib import ExitStack

    f32 = mybir.dt.float32
    bf16 = mybir.dt.bfloat16
    f8 = mybir.dt.float8e4
    DR = mybir.MatmulPerfMode.DoubleRow
    AF = mybir.ActivationFunctionType
    OP = mybir.AluOpType

    nc = bacc.Bacc("TRN2", target_bir_lowering=False, debug=False,
                   num_devices=NCORES)

    # ---- DRAM I/O ----
    x_d = nc.declare_dram_parameter("x", [T, E], bf16, isOutput=False)
    wq_d = nc.declare_dram_parameter("wq", [NJE, 128, E], f8, isOutput=False)
    wk_d = nc.declare_dram_parameter("wk", [NJE, 128, E], f8, isOutput=False)
    wv_d = nc.declare_dram_parameter("wv", [NJE, 128, E], f8, isOutput=False)
    wo_d = nc.declare_dram_parameter("wo", [NJE, 128, E], f8, isOutput=False)
    w1_d = nc.declare_dram_parameter("w1", [NJF, 128, E], bf16, isOutput=False)
    w2_d = nc.declare_dram_parameter("w2", [2, 128, NJF // 4, 2, 512], f8, isOutput=False)
    w2b_d = nc.declare_dram_parameter("w2b", [2, 128, NJF // 2, 512], bf16, isOutput=False)
    cq_d = nc.declare_dram_parameter("cq", [128, NJE], f32, isOutput=False)
    ck_d = nc.declare_dram_parameter("ck", [128, NJE], f32, isOutput=False)
    cvb_d = nc.declare_dram_parameter("cvb", [128, E], f32, isOutput=False)
    xq_d = nc.declare_dram_parameter("xq", [TQ, E], f32, isOutput=False)
    b2b_d = nc.declare_dram_parameter("b2b", [128, E], f32, isOutput=False)
    b1c_d = nc.declare_dram_parameter("b1c", [128, NJF], f32, isOutput=False)
    out_d = nc.declare_dram_parameter("out", [TQ, E], f32, isOutput=True)
    rbounce = nc.dram_tensor("rbounce", [H, TQ], f32)

    NSP = NMT // 2       # 8 key-tile pairs
    SC = float(HS) ** -0.5 / 64.0   # wq,wk each carry an 8x scale

    def layernorm(stats_pool, x_sb, out_bf, eps_sb, apply_on_act=True):
        st = stats_pool.tile([128, 2, 6], f32, name="ln_st")
        nc.vector.bn_stats(out=st[:, 0, :], in_=x_sb[:, 0:512])
        nc.vector.bn_stats(out=st[:, 1, :], in_=x_sb[:, 512:1024])
        mv = stats_pool.tile([128, 2], f32, name="ln_mv")
        nc.vector.bn_aggr(out=mv[:], in_=st[:])
        rstd = stats_pool.tile([128, 1], f32, name="ln_rstd")
        nc.scalar.activation(out=rstd[:], in_=mv[:, 1:2], func=AF.Sqrt,
                             bias=eps_sb[:])
        nc.vector.reciprocal(out=rstd[:], in_=rstd[:])
        nmr = stats_pool.tile([128, 1], f32, name="ln_nmr")
        nc.vector.tensor_tensor(out=nmr[:], in0=mv[:, 0:1], in1=rstd[:],
                                op=OP.mult)
        nc.vector.tensor_scalar_mul(out=nmr[:], in0=nmr[:], scalar1=-1.0)
        if apply_on_act:
            nc.scalar.activation(out=out_bf[:], in_=x_sb[:], func=AF.Identity,
                                 bias=nmr[:], scale=rstd[:])
        else:
            # keep the exp stream alone on ACT during attention passes
            nc.vector.tensor_scalar(out=out_bf[:], in0=x_sb[:],
                                    scalar1=rstd[:], scalar2=nmr[:],
                                    op0=OP.mult, op1=OP.add)

    with tile.TileContext(nc) as tc:
        top = ExitStack()

        # ---- constants + long-lived weights (left stack bottom) ----
        const = top.enter_context(tc.tile_pool(name="const", bufs=1, side="left"))
        xin = top.enter_context(tc.tile_pool(name="ln1x", bufs=6, side="left"))
        xt = {}

        def load_x(mt):
            # issued from inside the LN1 loop at a fixed prefetch distance:
            # issuing all 16 up-front head-blocks the sync queue (and the
            # transposes behind it) on the pool-recycle anti-deps
            x_sb = xin.tile([128, E], bf16)
            nc.sync.dma_start(out=x_sb[:], in_=x_d[mt * 128:(mt + 1) * 128, :])
            xt[mt] = x_sb
        for mt in range(4):
            load_x(mt)
        eps_sb = const.tile([128, 1], f32)
        nc.vector.memset(eps_sb[:], EPS)
        cq_sb = const.tile([128, NJE], f32)
        nc.sync.dma_start(out=cq_sb[:], in_=cq_d[:])
        ck_sb = const.tile([128, NJE], f32)
        nc.sync.dma_start(out=ck_sb[:], in_=ck_d[:])

        b2_sb = const.tile([128, E], f32)
        nc.sync.dma_start(out=b2_sb[:], in_=b2b_d[:])
        b1_sb = const.tile([128, NJF], f32)
        nc.sync.dma_start(out=b1_sb[:], in_=b1c_d[:])
        zero_sb = const.tile([128, 512], bf16)
        nc.vector.memset(zero_sb[:], 0.0)
        from concourse import masks
        ident = const.tile([128, 128], bf16)
        masks.make_identity(nc, ident[:])
        ones64 = const.tile([1, 64], f32)
        nc.vector.memset(ones64[:], 1.0)
        wo_sb = const.tile([128, NJE, E], f8)
        for j in range(NJE):
            nc.gpsimd.dma_start(out=wo_sb[:, j, :], in_=wo_d[j])

        # qkv weights: issued up-front from a pre-allocated pool so the loads
        # carry no SBUF-reuse anti-dependency on the LN1 pipeline
        w_es = ExitStack()
        wqkv = w_es.enter_context(tc.tile_pool(name="w_pool", bufs=1, side="left"))
        wq_sb = wqkv.tile([128, NJE, E], f8, name="wq")
        wk_sb = wqkv.tile([128, NJE, E], f8, name="wk")
        wv_sb = wqkv.tile([128, NJE, E], f8, name="wv")
        cv_sb = wqkv.tile([128, E], f32, name="cv")
        nc.sync.dma_start(out=cv_sb[:], in_=cvb_d[:])
        for j in range(NJE):
            nc.gpsimd.dma_start(out=wk_sb[:, j, :], in_=wk_d[j])
        for j in range(NJE):
            nc.gpsimd.dma_start(out=wq_sb[:, j, :], in_=wq_d[j])
        for j in range(NJE):
            nc.gpsimd.dma_start(out=wv_sb[:, j, :], in_=wv_d[j])

        # ---------- LN1 (transpose on the PE; emitted per-tile, woven into
        # pass A so the exp stream starts as soon as the first LN1 group
        # lands; a DMA-transpose in the per-tile chain serializes the DVE
        # queue behind multi-us DMA latencies) ----------
        hT_es = ExitStack()
        hT_pool = hT_es.enter_context(tc.tile_pool(name="hT", bufs=1, side="left"))
        h8 = [hT_pool.tile([128, NJE, 4, 128], f8, name=f"h8{g}")
              for g in range(4)]
        ln1_es = ExitStack()
        stp = ln1_es.enter_context(tc.tile_pool(name="ln1s", bufs=10, side="left"))
        hbp = ln1_es.enter_context(tc.tile_pool(name="ln1h", bufs=3, side="left"))

        # ---------- persistent QKV activations (right stack) ----------
        oT = top.enter_context(tc.tile_pool(name="oT", bufs=1, side="right")) \
            .tile([128, NJE, TQ], f8)
        qkv_es = ExitStack()
        qkv_pool = qkv_es.enter_context(
            tc.tile_pool(name="qkvact", bufs=1, side="right"))
        # q^T zero-padded per head (see scores matmul): head h in rows
        # (h%2)*64..+64 of [:, h, :], other 64 rows zero.
        # q/k live in f8: the scores matmul runs fp8 at the same 1 cy/col as
        # bf16, and the ~1% logit noise is far inside the error budget, so
        # this is 32KB of SBUF for free.
        qT = qkv_pool.tile([128, H, TQ], f8)
        kT = qkv_pool.tile([128, NJE, T], f8)
        # v (8x via wv scale) + 0.5-ones column: softmax sum = 0.5*sum(att),
        # so normalized attention lands at 16x natural scale (f8-friendly);
        # the proj epilogue folds in 1/128 (16x and the 8x wo scale).
        v_aug = qkv_pool.tile([128, NSP, 2, H, HS + 1], f8)
        for hh in range(H):
            p0 = 64 - (hh % 2) * 64
            nc.gpsimd.memset(qT[p0:p0 + 64, hh, :], 0.0)
        nc.vector.memset(v_aug[:, :, :, :, HS:HS + 1], 0.5)

        work_es = ExitStack()
        wps = work_es.enter_context(
            tc.tile_pool(name="work_ps", bufs=2, space="PSUM"))

        def emit_ln1(mt):
            if mt + 4 < NMT:
                load_x(mt + 4)
            h_bf = hbp.tile([128, E], bf16)
            layernorm(stp, xt[mt], h_bf, eps_sb)
            trp = wps.tile([128, E], bf16, name="ps_w")
            for j in range(NJE):
                nc.tensor.transpose(trp[:, j * 128:(j + 1) * 128],
                                    h_bf[:, j * 128:(j + 1) * 128], ident[:])
            g, sl = mt // 4, mt % 4
            nc.vector.tensor_copy(
                out=h8[g][:, 0:4, sl, :],
                in_=trp[:, 0:512].rearrange("p (j c) -> p j c", j=4))
            nc.scalar.activation(
                out=h8[g][:, 4:8, sl, :],
                in_=trp[:, 512:1024].rearrange("p (j c) -> p j c", j=4),
                func=AF.Copy)

        def emit_q(g, mf, pool):
                pq = pool.tile([128, 512], f32, name="ps_w")
                for j in range(0, NJE, 2):
                    nc.tensor.matmul(pq[:], wq_sb[:, j:j + 2, mf * 128:(mf + 1) * 128],
                                     h8[g][:, j:j + 2, :, :], perf_mode=DR,
                                     start=(j == 0), stop=(j == NJE - 2))
                sl = slice(g * 512, (g + 1) * 512)
                nc.vector.tensor_scalar_add(out=qT[0:64, 2 * mf, sl],
                                            in0=pq[0:64, :],
                                            scalar1=cq_sb[0:64, mf:mf + 1])
                nc.vector.tensor_scalar_add(out=qT[64:128, 2 * mf + 1, sl],
                                            in0=pq[64:128, :],
                                            scalar1=cq_sb[64:128, mf:mf + 1])

        def emit_k(g, mf, pool):
            pk = pool.tile([128, 512], f32, name="ps_w")
            for j in range(0, NJE, 2):
                nc.tensor.matmul(pk[:], wk_sb[:, j:j + 2, mf * 128:(mf + 1) * 128],
                                 h8[g][:, j:j + 2, :, :], perf_mode=DR,
                                 start=(j == 0), stop=(j == NJE - 2))
            nc.vector.tensor_scalar_add(out=kT[:, mf, g * 512:(g + 1) * 512],
                                        in0=pk[:],
                                        scalar1=ck_sb[:, mf:mf + 1])

        def emit_v(st, pool):
            pv0 = pool.tile([128, 512], f32, name="ps_w")
            pv1 = pool.tile([128, 512], f32, name="ps_w")
            for j in range(0, NJE, 2):
                lhsT = h8[st // 4][:, j:j + 2, st % 4, :]
                nc.tensor.matmul(pv0[:], lhsT, wv_sb[:, j:j + 2, 0:512],
                                 perf_mode=DR, start=(j == 0), stop=(j == NJE - 2))
                nc.tensor.matmul(pv1[:], lhsT, wv_sb[:, j:j + 2, 512:1024],
                                 perf_mode=DR, start=(j == 0), stop=(j == NJE - 2))
            nc.vector.tensor_tensor(
                out=v_aug[:, st // 2, st % 2, 0:8, 0:HS],
                in0=pv0.rearrange("p (h d) -> p h d", h=8),
                in1=cv_sb[:, 0:512].rearrange("p (h d) -> p h d", h=8),
                op=OP.add)
            nc.vector.tensor_tensor(
                out=v_aug[:, st // 2, st % 2, 8:16, 0:HS],
                in0=pv1.rearrange("p (h d) -> p h d", h=8),
                in1=cv_sb[:, 512:1024].rearrange("p (h d) -> p h d", h=8),
                op=OP.add)

        # Minimal pre-attention block. Scores consume kT slices in st order,
        # so head 0 only needs K(g0,mf0) before its first scores; later LN1
        # groups and their dependent K/V blocks weave into pass A.
        for mt in range(4):
            emit_ln1(mt)
        emit_k(0, 0, wps)
        emit_q(0, 0, wps)
        emit_v(0, wps)
        emit_v(1, wps)

        # ---------- attention (+ woven filler work) ----------
        att_es = ExitStack()
        aps = att_es.enter_context(tc.tile_pool(name="att_ps", bufs=2, space="PSUM"))
        ops = att_es.enter_context(tc.tile_pool(name="att_po", bufs=2, space="PSUM"))
        atp = att_es.enter_context(tc.tile_pool(name="att_t", bufs=2, side="right"))
        rp = att_es.enter_context(tc.tile_pool(name="att_r", bufs=4, side="right"))
        rbp = att_es.enter_context(tc.tile_pool(name="att_rb", bufs=2, side="right"))

        po_live = {}
        att_live = {}
        from collections import deque
        finish_q = deque()

        def emit_scores(p, h, sp):
            at2 = atp.tile([128, 2, 512], f8, name="att")
            ps = aps.tile([128, 2, 512], f32, name="ps_sc")
            qsl = qT[:, h, p * 512:(p + 1) * 512]
            for k2 in range(2):
                st = 2 * sp + k2
                nc.tensor.matmul(ps[:, k2, :],
                                 kT[:, h // 2, st * 128:(st + 1) * 128],
                                 qsl, start=True, stop=True)
            nc.scalar.activation(out=at2[:], in_=ps[:], func=AF.Exp, scale=SC)
            att_live[(h, sp)] = at2

        def emit_av(p, h, sp):
            if sp == 0:
                po_live[h] = ops.tile([HS + 1, 512], f32, name="ps_o")
            po = po_live[h]
            at2 = att_live.pop((h, sp))
            nc.tensor.matmul(po[:], v_aug[:, sp, :, h, :], at2[:],
                             perf_mode=DR,
                             start=(sp == 0), stop=(sp == NSP - 1))
            if sp == NSP - 1:
                emit_head_finish(p, h, po_live.pop(h))

        def emit_head_finish(p, h, po):
            # Copy the unnormalized head out of PSUM immediately (frees the
            # accumulator for head h+2) and broadcast 1/sum across the 64
            # partitions with a K=1 matmul on the tensor engine — a
            # DRAM-bounce broadcast here held po/att2 ~10us per head and
            # throttled the whole exp pipeline.
            sl = slice(p * 512, (p + 1) * 512)
            s1 = rp.tile([1, 512], f32, name="ssum")
            nc.vector.tensor_copy(out=s1[:], in_=po[HS:HS + 1, :])
            o_un = rbp.tile([64, 512], f32, name="oun")
            nc.vector.tensor_copy(out=o_un[:], in_=po[0:HS, :])
            r1 = rp.tile([1, 512], f32, name="rsum")
            nc.vector.reciprocal_approx_fast(out=r1[:], in_=s1[:])
            # the normalize-multiply is deferred a few steps so the K=1
            # broadcast matmul never waits on the recip at the head of the
            # tensor queue
            finish_q.append((p, h, o_un, r1))

        def flush_finish():
            while finish_q:
                p, h, o_un, r1 = finish_q.popleft()
                sl = slice(p * 512, (p + 1) * 512)
                rb = wps.tile([64, 512], f32, name="ps_w")
                nc.tensor.matmul(rb[:], ones64[:], r1[:], start=True, stop=True)
                p0 = (h % 2) * 64
                nc.vector.tensor_tensor(out=oT[p0:p0 + 64, h // 2, sl],
                                        in0=o_un[:], in1=rb[:], op=OP.mult)

        def run_pass(p, fillers):
            # fillers: list of (idx, closure) woven in after step idx
            fmap = {}
            for idx, fn in fillers:
                fmap.setdefault(idx, []).append(fn)
            steps = [(h, sp) for h in range(H) for sp in range(NSP)]
            for i, (h, sp) in enumerate(steps):
                emit_scores(p, h, sp)
                if i > 0:
                    emit_av(p, *steps[i - 1])
                if sp == 4:
                    flush_finish()
                for fn in fmap.get(i, []):
                    fn()
            emit_av(p, *steps[-1])
            flush_finish()
            for fn in fmap.get(len(steps), []):
                fn()

        # pass-A fillers: LN1 tiles, V pairs and K(g*,0) woven just ahead of
        # head 0's consumption; K(mf)+Q(g0,mf) during head 2mf-1; Q(g1,*)
        # (pass-B only) late.
        fillA = [(0, lambda: emit_ln1(4)), (0, lambda: emit_ln1(5)),
                 (0, lambda: emit_v(2, wps)), (0, lambda: emit_v(3, wps)),
                 (1, lambda: emit_ln1(6)), (1, lambda: emit_ln1(7)),
                 (1, lambda: emit_k(1, 0, wps)),
                 (2, lambda: emit_ln1(8)), (2, lambda: emit_ln1(9)),
                 (2, lambda: emit_v(4, wps)), (2, lambda: emit_v(5, wps)),
                 (3, lambda: emit_ln1(10)), (3, lambda: emit_ln1(11)),
                 (3, lambda: emit_v(6, wps)), (3, lambda: emit_v(7, wps)),
                 (3, lambda: emit_k(2, 0, wps)),
                 (4, lambda: emit_ln1(12)), (4, lambda: emit_ln1(13)),
                 (4, lambda: emit_v(8, wps)), (4, lambda: emit_v(9, wps)),
                 (5, lambda: emit_ln1(14)), (5, lambda: emit_ln1(15)),
                 (5, lambda: emit_k(3, 0, wps)),
                 (5, lambda: emit_v(10, wps)), (5, lambda: emit_v(11, wps)),
                 (6, lambda: emit_v(12, wps)), (6, lambda: emit_v(13, wps)),
                 (7, lambda: emit_v(14, wps)), (7, lambda: emit_v(15, wps))]
        for mf in range(1, NJE):
            base = 8 * (2 * mf - 1)
            for g in range(4):
                fillA.append((base + 2 * g,
                              (lambda gg, m: lambda: emit_k(gg, m, wps))(g, mf)))
            fillA.append((base + 7, (lambda m: lambda: emit_q(0, m, wps))(mf)))
        for mf in range(NJE):
            fillA.append((100 + 3 * mf, (lambda m: lambda: emit_q(1, m, wps))(mf)))
        run_pass(0, fillA)
        ln1_es.close()
        hT_es.close()
        w_es.close()

        # ---------- proj + LN2 + FFN emitters ----------
        ffn_es = ExitStack()
        pxp = ffn_es.enter_context(tc.tile_pool(name="proj_x", bufs=1, side="left"))
        xrp = ffn_es.enter_context(tc.tile_pool(name="xr", bufs=8, side="left"))
        h2p = ffn_es.enter_context(tc.tile_pool(name="h2T", bufs=1, side="left"))
        f1p8 = ffn_es.enter_context(tc.tile_pool(name="ffnT8", bufs=1, side="left"))
        f1pb = ffn_es.enter_context(tc.tile_pool(name="ffnTb", bufs=1, side="left"))
        w1p = ffn_es.enter_context(tc.tile_pool(name="f1w", bufs=2, side="left"))
        w2p8 = ffn_es.enter_context(tc.tile_pool(name="f2w8", bufs=1, side="left"))
        w2pb = ffn_es.enter_context(tc.tile_pool(name="f2wb", bufs=1, side="left"))
        stp2 = ffn_es.enter_context(tc.tile_pool(name="ln2s", bufs=6, side="left"))
        hbp2 = ffn_es.enter_context(tc.tile_pool(name="ln2h", bufs=1, side="left"))
        f2op = ffn_es.enter_context(tc.tile_pool(name="f2o", bufs=2, side="left"))
        # per-pass activation tiles cycle through bufs=1 pools: pass B reuses
        # pass A's buffer once the pass-A FFN (woven into attention pass B)
        # has consumed it
        h2Tb, ffnT8, ffnTb = {}, {}, {}
        xr_t = {}

        def emit_proj_ln2(p, mtl):
            if mtl == 0:
                h2Tb[p] = h2p.tile([128, NJE, 4, 128], bf16, name="h2T")
            mt = p * 4 + mtl
            x_sb = pxp.tile([128, E], f32, name="xq")
            nc.sync.dma_start(out=x_sb[:], in_=xq_d[mt * 128:(mt + 1) * 128, :])
            pa = wps.tile([128, 512], f32, name="ps_w")
            pb = wps.tile([128, 512], f32, name="ps_w")
            for j in range(0, NJE, 2):
                lhsT = oT[:, j:j + 2, mt * 128:(mt + 1) * 128]
                nc.tensor.matmul(pa[:], lhsT, wo_sb[:, j:j + 2, 0:512],
                                 perf_mode=DR, start=(j == 0), stop=(j == NJE - 2))
                nc.tensor.matmul(pb[:], lhsT, wo_sb[:, j:j + 2, 512:1024],
                                 perf_mode=DR, start=(j == 0), stop=(j == NJE - 2))
            xr = xrp.tile([128, E], f32, name="xr")
            xr_t[mt] = xr
            nc.vector.scalar_tensor_tensor(
                out=xr[:, 0:512], in0=pa[:], scalar=1.0 / 128.0,
                in1=x_sb[:, 0:512], op0=OP.mult, op1=OP.add)
            nc.vector.scalar_tensor_tensor(
                out=xr[:, 512:1024], in0=pb[:], scalar=1.0 / 128.0,
                in1=x_sb[:, 512:1024], op0=OP.mult, op1=OP.add)
            h_bf = hbp2.tile([128, E], bf16)
            layernorm(stp2, xr[:], h_bf, eps_sb, apply_on_act=False)
            nc.sync.dma_start_transpose(out=h2Tb[p][:, :, mtl, :], in_=h_bf[:])
            # after LN2 consumed xr, fold the output bias in place so the
            # FFN2 epilogue is a single op
            nc.vector.tensor_tensor(out=xr[:], in0=xr[:], in1=b2_sb[:],
                                    op=OP.add)

        def emit_ffn1(p, mf0, nmf):
            if mf0 == 0:
                ffnT8[p] = f1p8.tile([128, NJF // 2, 512], f8, name="fT8")
                ffnTb[p] = f1pb.tile([128, NJF // 2, 512], bf16, name="fTb")
            for mf in range(mf0, mf0 + nmf):
                w1_sb = w1p.tile([128, NJE, 128], bf16, name="w1t")
                nc.gpsimd.dma_start(
                    out=w1_sb[:],
                    in_=w1_d[mf].rearrange("p (j c) -> p j c", j=NJE))
                pf = wps.tile([128, 512], f32, name="ps_w")
                for j in range(NJE):
                    nc.tensor.matmul(pf[:], w1_sb[:, j, :],
                                     h2Tb[p][:, j, :, :],
                                     start=(j == 0), stop=(j == NJE - 1))
                dst = ffnT8[p][:, mf, :] if mf < NJF // 2 else \
                    ffnTb[p][:, mf - NJF // 2, :]
                # W1 carries the 8x scale (exact in bf16), so this single DVE
                # op yields 8x the true hidden without touching ACT
                nc.vector.scalar_tensor_tensor(
                    out=dst, in0=pf[:], scalar=b1_sb[:, mf:mf + 1],
                    in1=zero_sb[:], op0=OP.add, op1=OP.max)

        def emit_ffn2_w(nbh):
            w2_sb = w2p8.tile([128, NJF // 4, 2, 512], f8, name="w2t")
            nc.gpsimd.dma_start(out=w2_sb[:], in_=w2_d[nbh])
            w2b_sb = w2pb.tile([128, NJF // 2, 512], bf16, name="w2bt")
            nc.gpsimd.dma_start(out=w2b_sb[:], in_=w2b_d[nbh])
            return w2_sb, w2b_sb

        def emit_ffn2(p, nbh, w2_sb, w2b_sb, tps, mtls=(0, 1, 2, 3)):
            psums = {}
            for mtl in mtls:
                mt = p * 4 + mtl
                psums[mt] = tps.tile([128, 512], f32, name="ps_w")
            for kp in range(NJF // 4):
                for mtl in mtls:
                    mt = p * 4 + mtl
                    nc.tensor.matmul(psums[mt][:],
                                     ffnT8[p][:, 2 * kp:2 * kp + 2, mtl * 128:(mtl + 1) * 128],
                                     w2_sb[:, kp, :, :], perf_mode=DR,
                                     start=(kp == 0), stop=False)
            for k in range(NJF // 2):
                for mtl in mtls:
                    mt = p * 4 + mtl
                    nc.tensor.matmul(psums[mt][:],
                                     ffnTb[p][:, k, mtl * 128:(mtl + 1) * 128],
                                     w2b_sb[:, k, :],
                                     start=False, stop=(k == NJF // 2 - 1))
            for mt, ps2 in psums.items():
                o_sb = f2op.tile([128, 512], f32, name="osb")
                nc.vector.scalar_tensor_tensor(
                    out=o_sb[:], in0=ps2[:], scalar=1.0 / 64.0,
                    in1=xr_t[mt][:, nbh * 512:(nbh + 1) * 512],
                    op0=OP.mult, op1=OP.add)
                nc.sync.dma_start(
                    out=out_d[mt * 128:(mt + 1) * 128,
                              nbh * 512:(nbh + 1) * 512],
                    in_=o_sb[:])

        # pass B with pass-A proj/LN2/FFN woven in
        fillB = []
        for mtl in range(4):
            fillB.append((2 + 3 * mtl, (lambda m: lambda: emit_proj_ln2(0, m))(mtl)))
        for c in range(16):
            fillB.append((16 + 3 * c,
                          (lambda c0: lambda: emit_ffn1(0, 2 * c0, 2))(c)))
        w2h = {}

        def load_w2h(nbh):
            w2h[nbh] = emit_ffn2_w(nbh)
        fillB.append((88, lambda: load_w2h(0)))
        fillB.append((96, lambda: emit_ffn2(0, 0, *w2h[0], wps, (0, 1))))
        fillB.append((104, lambda: emit_ffn2(0, 0, *w2h[0], wps, (2, 3))))
        fillB.append((108, lambda: load_w2h(1)))
        fillB.append((114, lambda: emit_ffn2(0, 1, *w2h[1], wps, (0, 1))))
        fillB.append((121, lambda: emit_ffn2(0, 1, *w2h[1], wps, (2, 3))))
        run_pass(1, fillB)
        att_es.close()
        qkv_es.close()

        # ---------- tail ----------
        # proj/LN2-B first so their DVE/ACT chains run under FFN2-A's
        # matmuls; FFN1-B then finds h2Tb ready.
        tail_es = ExitStack()
        tps = tail_es.enter_context(tc.tile_pool(name="f2ps", bufs=4, space="PSUM"))
        for mtl in range(4):
            emit_proj_ln2(1, mtl)
        emit_ffn1(1, 0, NJF)
        for nbh in range(2):
            w2_sb, w2b_sb = emit_ffn2_w(nbh)
            emit_ffn2(1, nbh, w2_sb, w2b_sb, tps)

        tail_es.close()
        ffn_es.close()
        work_es.close()
        top.close()

    nc.compile()
    return nc


def _prep_weights(ln1_g, ln1_b, Wq, Wk, Wv, Wo, bo, ln2_g, ln2_b, W1, b1, W2, b2):
    f64 = np.float64
    g1 = np.asarray(ln1_g, f64)
    b1ln = np.asarray(ln1_b, f64)
    g2 = np.asarray(ln2_g, f64)
    b2ln = np.asarray(ln2_b, f64)

    def flat_qkv(W):
        return np.asarray(W, f64).transpose(1, 0, 2).reshape(E, H * HS)

    Wqf, Wkf, Wvf = flat_qkv(Wq), flat_qkv(Wk), flat_qkv(Wv)
    out = {}
    # All f8 weights carry an 8x scale so sigma~0.02 values clear the e4m3
    # subnormal floor; the kernel folds the compensating scales into the exp
    # (1/64), the proj epilogue (1/128, including the 16x from the 0.5 ones
    # column), and the FFN2 epilogue (1/64).
    out["wq"] = np.ascontiguousarray((8 * g1[:, None] * Wqf).reshape(NJE, 128, E).astype(F8))
    out["wk"] = np.ascontiguousarray((8 * g1[:, None] * Wkf).reshape(NJE, 128, E).astype(F8))
    out["wv"] = np.ascontiguousarray((8 * g1[:, None] * Wvf).reshape(NJE, 128, E).astype(F8))
    cq = (8 * b1ln @ Wqf).astype(np.float32)
    ck = (8 * b1ln @ Wkf).astype(np.float32)
    cv = (8 * b1ln @ Wvf).astype(np.float32)
    out["cq"] = np.ascontiguousarray(cq.reshape(NJE, 128).T)
    out["ck"] = np.ascontiguousarray(ck.reshape(NJE, 128).T)
    out["cvb"] = np.ascontiguousarray(np.broadcast_to(cv, (128, E)))
    out["wo"] = np.ascontiguousarray(
        (8 * np.asarray(Wo, f64)).reshape(NJE, 128, E).astype(F8))
    W1p = 8 * g2[:, None] * np.asarray(W1, f64)
    b1p = (8 * (np.asarray(b1, f64) + b2ln @ np.asarray(W1, f64))).astype(np.float32)
    out["w1"] = np.ascontiguousarray(
        W1p.reshape(NJE, 128, NJF, 128).transpose(2, 1, 0, 3).reshape(NJF, 128, E).astype(BF))
    out["b1c"] = np.ascontiguousarray(b1p.reshape(NJF, 128).T)
    w2s = (8 * np.asarray(W2, f64)).reshape(NJF, 128, 2, 512)
    out["w2"] = np.ascontiguousarray(
        w2s[:NJF // 2].reshape(NJF // 4, 2, 128, 2, 512)
        .transpose(3, 2, 0, 1, 4).astype(F8))
    out["w2b"] = np.ascontiguousarray(
        w2s[NJF // 2:].transpose(2, 1, 0, 3).astype(BF))
    out["b2b"] = np.ascontiguousarray(
        np.broadcast_to(np.asarray(b2, np.float32), (128, E)))
    return out


def kernel(x, ln1_g, ln1_b, Wq, Wk, Wv, Wo, bo, ln2_g, ln2_b, W1, b1, W2, b2):
    global LAST_RESULTS
    from concourse.bass_utils import run_bass_kernel_spmd

    if "nc" not in _CACHE:
        _CACHE["nc"] = _build()
    nc = _CACHE["nc"]

    wmap = _prep_weights(ln1_g, ln1_b, Wq, Wk, Wv, Wo, bo,
                         ln2_g, ln2_b, W1, b1, W2, b2)
    x = np.asarray(x, np.float32)

    in_maps = []
    for c in range(NCORES):
        b, half = c // 2, c % 2
        xb = x[b]
        x_roll = np.ascontiguousarray(
            np.concatenate([xb[half * TQ:], xb[:half * TQ]], axis=0))
        m = dict(wmap)
        m["x"] = x_roll.astype(BF)
        m["xq"] = np.ascontiguousarray(
            x_roll[:TQ] + np.asarray(bo, np.float32)[None, :])
        in_maps.append(m)

    res = run_bass_kernel_spmd(nc, in_maps, list(range(NCORES)), trace=TRACE)
    LAST_RESULTS = res

    out = np.empty((B, T, E), np.float32)
    for c in range(NCORES):
        b, half = c // 2, c % 2
        out[b, half * TQ:(half + 1) * TQ] = res.results[c]["out"]
    return out

